# revision 8
# baseline (speedup 1.0000x reference)
"""AlphaRenderer kernel for 8 TRN2 NeuronCores.

Reference computation (per character n of N=4096):
    out[n] = sum_{k in top20 fonts of its text row} softmax_w[k] * alpha_table[font_k, char_class_n] / 255

Rewritten as a dense matmul over all 100 fonts with a top-20-masked
softmax weight matrix:
    out[n] = W[ti[n], :] @ alpha_table[:, c_n, :, :]        (W zero outside top-20)

Sharding strategy: shard by CHAR CLASS. Characters are grouped by their
argmax char class; each class group becomes a [<=KCAP, 100] weight block
that multiplies that class's [100, 4096] alpha-table slice. The 164 MB
table is then read exactly once across the chip (vs 8.2x for the naive
per-character gather). Class chunks are packed into S slots per core;
the host prepares, per core:
    table  [S, 100, 4096]  bf16  - alpha slice per slot (raw 0..255 vals)
    lhsT   [100, S*KCAP]   bf16  - transposed weights (softmax/255, masked)
and the device computes out[s*KCAP:(s+1)*KCAP] = lhsT_s.T @ table_s
via TensorEngine matmuls (K=100, M=KCAP, N=512 tiles), PSUM -> SBUF
bf16 cast, and streams results back. The host scatters rows back to the
original character order.
"""
import ml_dtypes
import numpy as np

BF16 = np.dtype(ml_dtypes.bfloat16)

import concourse.bass as bass
import concourse.mybir as mybir
import concourse.tile as tile
from concourse import bacc
from concourse.bass_utils import run_bass_kernel_spmd

NCORES = 8
F = 100          # fonts
FP = 112         # fonts padded for DMA balance (A/B: 112 vs 128)
C = 100          # char classes
N = 4096         # characters
HW = 4096        # 64*64 pixels
TOPK = 20
KCAP = 64        # rows per class slot (seed-0 max class count is 56)
NT = 512         # matmul free-dim tile (one PSUM bank of fp32)

_NC_CACHE: dict = {}
LAST_RESULT = None


def _build(S: int):
    """Per-core SPMD graph: S class slots of KCAP output rows each."""
    if S in _NC_CACHE:
        return _NC_CACHE[S]
    dt = mybir.dt.bfloat16
    nc = bacc.Bacc("TRN2", target_bir_lowering=False, debug=False,
                   num_devices=NCORES)
    table = nc.dram_tensor("table", [S, FP, HW], dt, kind="ExternalInput").ap()
    lhsT = nc.dram_tensor("lhsT", [FP, S * KCAP], dt, kind="ExternalInput").ap()
    out = nc.dram_tensor("out", [S * KCAP, HW], dt, kind="ExternalOutput").ap()

    PS = 1024        # psum tile free dim: 2 banks, 1 full-width copy
    HALF = HW // 2   # table loads split in two so matmuls start earlier
    with tile.TileContext(nc) as tc:
        with tc.tile_pool(name="w", bufs=1) as wpool, \
             tc.tile_pool(name="rhs", bufs=16) as rpool, \
             tc.tile_pool(name="ps", bufs=4, space="PSUM") as ppool, \
             tc.tile_pool(name="ot", bufs=3) as opool:
            wt = wpool.tile([FP, S * KCAP], dt)
            nc.gpsimd.dma_start(wt[:], lhsT[:])
            # Pair slots: slot pair (2p, 2p+1) writes partitions 0-63 /
            # 64-127 of shared PSUM tiles (col-tiled matmuls), so the
            # PSUM->SBUF cast copies and the output DMAs run the full
            # 128-partition width. Input DMAs issue from the Sync HWDGE
            # queue; output DMAs from GpSimd SWDGE so they cannot
            # head-of-line-block the table stream.
            for p in range((S + 1) // 2):
                nslots = min(2, S - 2 * p)
                ot = opool.tile([64 * nslots, HW], dt, tag="ot")
                rts = []
                for h in range(nslots):
                    halves = []
                    for j in range(2):
                        rt = rpool.tile([FP, HALF], dt, tag="rhs")
                        nc.sync.dma_start(
                            rt[:], table[2 * p + h, :, j * HALF:(j + 1) * HALF])
                        halves.append(rt)
                    rts.append(halves)
                for c in range(HW // PS):
                    pt = ppool.tile([64 * nslots, PS], mybir.dt.float32,
                                    tag="ps")
                    for h in range(nslots):
                        s = 2 * p + h
                        for n in range(PS // NT):
                            col = c * PS + n * NT
                            nc.tensor.matmul(
                                pt[h * 64:h * 64 + 64, n * NT:(n + 1) * NT],
                                wt[:, s * KCAP:(s + 1) * KCAP],
                                rts[h][col // HALF][:, col % HALF:
                                                    col % HALF + NT],
                                start=True, stop=True,
                                tile_position=(0, 64 * h) if nslots == 2
                                else None,
                            )
                    nc.any.tensor_copy(ot[:, c * PS:(c + 1) * PS], pt[:])
                nc.gpsimd.dma_start(out[p * 128:p * 128 + 64 * nslots, :],
                                    ot[:])
    nc.compile()
    _NC_CACHE[S] = nc
    return nc


def kernel(font_pred, char_labels, char_rec_vec, text_indexes, alpha_table):
    global LAST_RESULT
    BT = font_pred.shape[0] * font_pred.shape[1]

    # --- host: masked-softmax weight matrix [BT, F] ---
    fp = np.asarray(font_pred, np.float32).reshape(BT, F)
    m = fp.max(axis=1, keepdims=True)
    e = np.exp(fp - m)
    sfm = e / e.sum(axis=1, keepdims=True)
    topk = np.argpartition(-fp, TOPK - 1, axis=1)[:, :TOPK]
    M = np.zeros((BT, F), np.float32)
    rows = np.arange(BT)[:, None]
    M[rows, topk] = sfm[rows, topk]
    M *= np.float32(1.0 / 255.0)

    char_idx = np.asarray(char_rec_vec).argmax(axis=1)
    ti = np.asarray(text_indexes).reshape(-1)
    Wc = M[ti]                                   # [N, F] per-char weights

    # --- host: group chars by class, chunk to <=KCAP, pack into cores ---
    chunks = []                                  # (class, np.array(char_ids))
    order = np.argsort(char_idx, kind="stable")
    sorted_cls = char_idx[order]
    starts = np.searchsorted(sorted_cls, np.arange(C), side="left")
    ends = np.searchsorted(sorted_cls, np.arange(C), side="right")
    for c in range(C):
        ids = order[starts[c]:ends[c]]
        for i in range(0, len(ids), KCAP):
            chunks.append((c, ids[i:i + KCAP]))
    S = max(1, -(-len(chunks) // NCORES))
    per_core = [chunks[i::NCORES] for i in range(NCORES)]

    tbl = np.asarray(alpha_table, np.float32).reshape(F, C, HW)
    tbl_bf = tbl.astype(BF16)

    in_maps = []
    slot_ids = []                                # per core, per slot char ids
    for core in range(NCORES):
        table_i = np.zeros((S, FP, HW), BF16)
        lhsT_i = np.zeros((FP, S * KCAP), np.float32)
        ids_i = []
        for s, (c, ids) in enumerate(per_core[core]):
            table_i[s, :F] = tbl_bf[:, c, :]
            lhsT_i[:F, s * KCAP:s * KCAP + len(ids)] = Wc[ids].T
            ids_i.append(ids)
        in_maps.append({"table": table_i,
                        "lhsT": lhsT_i.astype(BF16)})
        slot_ids.append(ids_i)

    nc = _build(S)
    res = run_bass_kernel_spmd(nc, in_maps, core_ids=list(range(NCORES)))
    LAST_RESULT = res

    out_full = np.zeros((N, HW), np.float32)
    for core in range(NCORES):
        o = np.asarray(res.results[core]["out"], np.float32)
        for s, ids in enumerate(slot_ids[core]):
            out_full[ids] = o[s * KCAP:s * KCAP + len(ids)]
    return out_full.reshape(N, 1, 1, 64, 64)


# revision 10
# speedup vs baseline: 1.0988x; 1.0988x over previous
"""AlphaRenderer kernel for 8 TRN2 NeuronCores.

Reference computation (per character n of N=4096):
    out[n] = sum_{k in top20 fonts of its text row} softmax_w[k] * alpha_table[font_k, char_class_n] / 255

Rewritten as a dense matmul over all 100 fonts with a top-20-masked
softmax weight matrix:
    out[n] = W[ti[n], :] @ alpha_table[:, c_n, :, :]        (W zero outside top-20)

Sharding strategy: shard by CHAR CLASS. Characters are grouped by their
argmax char class; each class group becomes a [<=KCAP, 100] weight block
that multiplies that class's [100, 4096] alpha-table slice. The 164 MB
table is then read exactly once across the chip (vs 8.2x for the naive
per-character gather). Class chunks are packed into S slots per core.

Device layout choices (all DMA-bandwidth driven):
  - fonts padded 100->128 partitions: the HWDGE splits a DMA into
    contiguous partition blocks, and only P=128 aligns the 16 SDMA
    engines with their SBUF ports.
  - slot PAIRS share PSUM tiles via column-tiled matmuls
    (tile_position=(0,64)), so PSUM->SBUF casts and output DMAs run
    the full 128-partition width.
  - the host interleaves each pair's two table slices row-wise
    ([128, 2*4096] bf16 = 16 KB contiguous per partition) and the
    output groups two pairs the same way: bigger DMA descriptors,
    better per-engine bandwidth.
  - input stream on Sync HWDGE, output stream on GpSimd SWDGE so
    output DMAs never head-of-line-block the table stream.
"""
import ml_dtypes
import numpy as np

import concourse.mybir as mybir
import concourse.tile as tile
from concourse import bacc
from concourse.bass_utils import run_bass_kernel_spmd

BF16 = np.dtype(ml_dtypes.bfloat16)

NCORES = 8
F = 100          # fonts
FP = 128         # fonts padded to full partition width
C = 100          # char classes
N = 4096         # characters
HW = 4096        # 64*64 pixels
TOPK = 20
KCAP = 64        # rows per class slot (seed-0 max class count is 56)
NT = 512         # matmul free-dim tile (one PSUM bank of fp32)
PS = 1024        # psum tile free dim: 2 banks, 1 full-width copy

_NC_CACHE: dict = {}
LAST_RESULT = None


def _build(S: int):
    """Per-core SPMD graph: S class slots of KCAP output rows each."""
    if S in _NC_CACHE:
        return _NC_CACHE[S]
    dt = mybir.dt.bfloat16
    npairs = (S + 1) // 2
    ngrp = (npairs + 1) // 2
    nc = bacc.Bacc("TRN2", target_bir_lowering=False, debug=False,
                   num_devices=NCORES)
    table = nc.dram_tensor("table", [npairs, FP, 2 * HW], dt,
                           kind="ExternalInput").ap()
    lhsT = nc.dram_tensor("lhsT", [FP, S * KCAP], dt,
                          kind="ExternalInput").ap()
    out = nc.dram_tensor("out", [ngrp, 128, 2 * HW], dt,
                         kind="ExternalOutput").ap()

    with tile.TileContext(nc) as tc:
        with tc.tile_pool(name="w", bufs=1) as wpool, \
             tc.tile_pool(name="rhs", bufs=4) as rpool, \
             tc.tile_pool(name="ps", bufs=4, space="PSUM") as ppool, \
             tc.tile_pool(name="og", bufs=2) as opool:
            wt = wpool.tile([FP, S * KCAP], dt)
            nc.sync.dma_start(wt[:], lhsT[:])
            og = None
            for p in range(npairs):
                nslots = min(2, S - 2 * p)
                width = HW * nslots
                if p % 2 == 0:
                    og = opool.tile([128, 2 * HW], dt, tag="og")
                rt = rpool.tile([FP, 2 * HW], dt, tag="rhs")
                nc.sync.dma_start(rt[:, :width], table[p, :, :width])
                ocol = (p % 2) * HW
                for c in range(HW // PS):
                    pt = ppool.tile([64 * nslots, PS], mybir.dt.float32,
                                    tag="ps")
                    for h in range(nslots):
                        s = 2 * p + h
                        for n in range(PS // NT):
                            col = h * HW + c * PS + n * NT
                            nc.tensor.matmul(
                                pt[h * 64:h * 64 + 64, n * NT:(n + 1) * NT],
                                wt[:, s * KCAP:(s + 1) * KCAP],
                                rt[:, col:col + NT],
                                start=True, stop=True,
                                tile_position=(0, 64 * h) if nslots == 2
                                else None,
                            )
                    nc.any.tensor_copy(
                        og[:64 * nslots, ocol + c * PS:ocol + (c + 1) * PS],
                        pt[:])
                if p % 2 == 1 or p == npairs - 1:
                    gwidth = HW * (2 if p % 2 == 1 else 1)
                    nc.gpsimd.dma_start(out[p // 2, :, :gwidth],
                                        og[:, :gwidth])
    nc.compile()
    _NC_CACHE[S] = nc
    return nc


def kernel(font_pred, char_labels, char_rec_vec, text_indexes, alpha_table):
    global LAST_RESULT
    BT = font_pred.shape[0] * font_pred.shape[1]

    # --- host: masked-softmax weight matrix [BT, F] ---
    fp = np.asarray(font_pred, np.float32).reshape(BT, F)
    m = fp.max(axis=1, keepdims=True)
    e = np.exp(fp - m)
    sfm = e / e.sum(axis=1, keepdims=True)
    topk = np.argpartition(-fp, TOPK - 1, axis=1)[:, :TOPK]
    M = np.zeros((BT, F), np.float32)
    rows = np.arange(BT)[:, None]
    M[rows, topk] = sfm[rows, topk]
    M *= np.float32(1.0 / 255.0)

    char_idx = np.asarray(char_rec_vec).argmax(axis=1)
    ti = np.asarray(text_indexes).reshape(-1)
    Wc = M[ti]                                   # [N, F] per-char weights

    # --- host: group chars by class, chunk to <=KCAP, pack into cores ---
    chunks = []                                  # (class, np.array(char_ids))
    order = np.argsort(char_idx, kind="stable")
    sorted_cls = char_idx[order]
    starts = np.searchsorted(sorted_cls, np.arange(C), side="left")
    ends = np.searchsorted(sorted_cls, np.arange(C), side="right")
    for c in range(C):
        ids = order[starts[c]:ends[c]]
        for i in range(0, len(ids), KCAP):
            chunks.append((c, ids[i:i + KCAP]))
    S = max(1, -(-len(chunks) // NCORES))
    npairs = (S + 1) // 2
    per_core = [chunks[i::NCORES] for i in range(NCORES)]

    tbl = np.asarray(alpha_table, np.float32).reshape(F, C, HW)
    tbl_bf = tbl.astype(BF16)

    in_maps = []
    slot_ids = []                                # per core, per slot char ids
    for core in range(NCORES):
        table_i = np.zeros((npairs, FP, 2, HW), BF16)
        lhsT_i = np.zeros((FP, S * KCAP), np.float32)
        ids_i = []
        for s, (c, ids) in enumerate(per_core[core]):
            table_i[s // 2, :F, s % 2] = tbl_bf[:, c, :]
            lhsT_i[:F, s * KCAP:s * KCAP + len(ids)] = Wc[ids].T
            ids_i.append(ids)
        in_maps.append({"table": table_i.reshape(npairs, FP, 2 * HW),
                        "lhsT": lhsT_i.astype(BF16)})
        slot_ids.append(ids_i)

    nc = _build(S)
    res = run_bass_kernel_spmd(nc, in_maps, core_ids=list(range(NCORES)))
    LAST_RESULT = res

    out_full = np.zeros((N, HW), np.float32)
    for core in range(NCORES):
        o = np.asarray(res.results[core]["out"], np.float32)
        for s, ids in enumerate(slot_ids[core]):
            p, h = divmod(s, 2)
            g, q = divmod(p, 2)
            rows = o[g, h * 64:h * 64 + len(ids), q * HW:(q + 1) * HW]
            out_full[ids] = rows
    return out_full.reshape(N, 1, 1, 64, 64)
